# revision 36
# baseline (speedup 1.0000x reference)
"""CuPyLinear (sparse CSR y = x @ W.T) Trainium2 kernel.

Problem shapes (hardcoded per spec):
  x       [512, 2048] f32
  data    [262144]    f32   (2048 rows x 128 nnz/row, uniform)
  indices [262144]    i32   (sorted per row, duplicates sum)
  indptr  [2049]      i32   (= arange*128, uniform -> unused on device)
  out y   [512, 2048] f32

Sharding: replicate x, shard the 2048 output rows across 8 cores
(256 rows each). Per core:
  1. segmented-scan dedupe of sorted per-row indices (duplicates sum)
  2. densify W rows in fp16 via gpsimd local_scatter (three pieces per
     row tile, each in its own tile for precise dependencies)
  3. transpose W.T with PE identity matmuls (batched through fp16 PSUM,
     PSUM->SBUF copies alternating ACT/DVE)
  4. y.T = W @ x.T as one fp16 matmul set per row tile (f32 PSUM accum)
     End-to-end rel err ~3e-4 (fp16 quantization of W and x).
Host gathers the 8 row-shards of y.T and transposes.
"""

import os
import sys

sys.path.insert(0, "/opt/trn_rl_repo")

from contextlib import ExitStack

import ml_dtypes
import numpy as np

import concourse.bass as bass
import concourse.tile as tile
from concourse import bacc, mybir
from concourse.bass_utils import run_bass_kernel_spmd

P = 128          # partitions
OUT = 2048       # out features (rows of sparse W)
IN = 2048        # in features (cols of sparse W)
N = 512          # tokens
J = 128          # nnz per row (uniform)
NCORES = 8
R_PER_CORE = OUT // NCORES   # 256
RT = R_PER_CORE // P         # 2 row-tiles per core
CT = IN // P                 # 16 contraction tiles
# W is scattered in three pieces per row tile, ordered so the first piece
# has the shortest index-computation path (subtract only) and the last
# piece is small (short critical tail). local_scatter num_elems < 2048.
PIECES = ((1536, 512), (0, 1024), (1024, 512))

BF16 = ml_dtypes.bfloat16
F32 = mybir.dt.float32
BF = mybir.dt.bfloat16
FP16 = mybir.dt.float16
I16 = mybir.dt.int16


def build_program():
    """Build + compile the per-core Bass program (same program on all cores)."""
    nc = bacc.Bacc("TRN2", target_bir_lowering=False, debug=False)

    xt_d = nc.dram_tensor("xt", [P, CT, N], FP16, kind="ExternalInput").ap()
    ident_d = nc.dram_tensor("ident", [P, P], FP16, kind="ExternalInput").ap()
    cv_d = nc.dram_tensor("cv", [P, 2, RT, J], F32, kind="ExternalInput").ap()
    yt_d = nc.dram_tensor("yt", [RT, P, N], F32, kind="ExternalOutput").ap()

    with tile.TileContext(nc) as tc, ExitStack() as ctx:
        const = ctx.enter_context(tc.tile_pool(name="const", bufs=1))
        xpool = ctx.enter_context(tc.tile_pool(name="x", bufs=1))
        work = ctx.enter_context(tc.tile_pool(name="work", bufs=2))
        wpool = ctx.enter_context(tc.tile_pool(name="w", bufs=2))
        psum_t = ctx.enter_context(tc.tile_pool(name="psum_t", bufs=4, space="PSUM"))
        psum_w = ctx.enter_context(tc.tile_pool(name="psum_w", bufs=1, space="PSUM"))
        psum_y = ctx.enter_context(tc.tile_pool(name="psum_y", bufs=2, space="PSUM"))
        ypool = ctx.enter_context(tc.tile_pool(name="y", bufs=2))

        # resident dedupe inputs first so DVE/Pool work starts immediately;
        # the big x tiles stream in behind them.
        cv_sb = xpool.tile([P, 2, RT, J], F32)
        nc.sync.dma_start(cv_sb[:], cv_d[:])
        ident = const.tile([P, P], FP16)
        nc.sync.dma_start(ident[:], ident_d[:])
        xf = xpool.tile([P, CT, N], FP16)
        XCHUNK = CT // 4
        # chunk order matches matmul ct consumption order (piece C first)
        for xc in (12, 0, 4, 8):
            nc.sync.dma_start(
                xf[:, xc : xc + XCHUNK, :], xt_d[:, xc : xc + XCHUNK, :]
            )

        # PE p-state warmup: the tensor engine needs ~3us of continuous
        # work to reach full clock. Chained dummy transposes of the identity
        # keep it busy from when `ident` lands until the first real
        # transposes are ready, so real work runs warm from the start.
        warm = psum_w.tile([P, P], FP16, space="PSUM", tag="warm")
        for _ in range(16):
            nc.tensor.transpose(warm[:], ident[:], ident[:])

        # ---- stage 1: dedupe (segmented scan over sorted cols), one pass
        # per row tile ([128, J] ops; rt0's chain finishes sooner and rt1's
        # overlaps rt0's scatters). Per-rt tiles keep dependencies precise.
        negone = const.tile([P, J], F32)
        nc.vector.memset(negone[:], -1.0)
        s16s = []
        piece_idxs = []
        for rt in range(RT):
            j0 = rt * J
            C = cv_sb[:, 0, rt, :]
            V = cv_sb[:, 1, rt, :]
            # eq[j] = (c[j] == c[j-1]); eq[0] = 0
            eq = work.tile([P, J], F32, tag=f"eq{rt}")
            nc.vector.memset(eq[:, 0:1], 0.0)
            nc.vector.tensor_tensor(
                eq[:, 1:J], C[:, 1:J], C[:, 0 : J - 1], op=mybir.AluOpType.is_equal
            )
            # segmented inclusive sum: s[j] = eq[j]*s[j-1] + v[j]
            s = work.tile([P, J], F32, tag=f"s{rt}")
            nc.vector.tensor_tensor_scan(
                s[:], eq[:], V, 0.0,
                op0=mybir.AluOpType.mult, op1=mybir.AluOpType.add,
            )
            # islast[j] = (c[j] != c[j+1]); islast[J-1] = 1
            islast = work.tile([P, J], mybir.dt.uint8, tag=f"il{rt}")
            nc.vector.memset(islast[:, J - 1 : J], 1)
            nc.vector.tensor_tensor(
                islast[:, 0 : J - 1], C[:, 0 : J - 1], C[:, 1:J],
                op=mybir.AluOpType.not_equal,
            )
            # keep col index only at last-of-run, else -1
            idxk = work.tile([P, J], F32, tag=f"idxk{rt}")
            nc.vector.select(idxk[:], islast[:], C, negone[:])
            # per-piece indices: keep idx-lo when lo <= idx < hi, else
            # negative (ignored by local_scatter)
            pidx = []
            for pi, (lo, width) in enumerate(PIECES):
                hi = lo + width
                if hi < IN:
                    m = work.tile([P, J], mybir.dt.uint8, tag=f"m{pi}_{rt}")
                    nc.vector.tensor_scalar(
                        m[:], idxk[:], float(hi), None, op0=mybir.AluOpType.is_lt
                    )
                    t = work.tile([P, J], F32, tag=f"t{pi}_{rt}")
                    nc.vector.select(t[:], m[:], idxk[:], negone[:])
                else:
                    t = idxk
                ip = work.tile([P, J], I16, tag=f"i{pi}_{rt}")
                if lo > 0:
                    # subtract fused with the int16 cast on the output
                    nc.vector.tensor_scalar_add(ip[:], t[:], -float(lo))
                else:
                    nc.vector.tensor_copy(ip[:], t[:])
                pidx.append(ip)
            piece_idxs.append(pidx)
            # scatter values in fp16 (11-bit mantissa; e2e error ~3e-4)
            s16 = work.tile([P, J], FP16, tag=f"s16{rt}")
            nc.vector.tensor_copy(s16[:], s[:])
            s16s.append(s16)

        # ---- stage 2: densify W rows via local_scatter (r-part, c-free),
        # then PE identity-transposes each piece (which also keeps the PE
        # p-state ramped before the matmuls), 4 [128,128] blocks per fp16
        # PSUM bank, PSUM->SBUF copies alternating ACT/DVE. wtf[pi, po, r]
        # holds W.T row c = po*128 + pi -> the [c-part, ct, r] lhsT layout.
        QCT = CT // 4
        wtf32 = []
        prev_scatter = None
        from concourse.tile import add_dep_helper
        for rt in range(RT):
            j0 = rt * J
            wtf = wpool.tile([P, CT, P], FP16, tag="wtf")
            for pi, ((c0, width), idx) in enumerate(zip(PIECES, piece_idxs[rt])):
                bt0 = c0 // P
                nblk = width // P
                # each piece scatters into its own tile: precise dependency
                # so this piece's transposes start as soon as IT is done
                wp = wpool.tile([P, width], FP16, tag=f"wp{pi}")
                sc = nc.gpsimd.local_scatter(
                    wp[:],
                    s16s[rt][:],
                    idx[:],
                    channels=P,
                    num_elems=width,
                    num_idxs=J,
                )
                # pin Pool order to emission order (so the small final piece
                # gives a short critical tail)
                if prev_scatter is not None:
                    add_dep_helper(sc.ins, prev_scatter.ins, sync=False)
                prev_scatter = sc
                for q0 in range(0, nblk, QCT):
                    qn = min(QCT, nblk - q0)
                    pt = psum_t.tile([P, QCT, P], FP16, space="PSUM", tag="pt")
                    for b in range(qn):
                        blk = (q0 + b) * P
                        nc.tensor.transpose(
                            pt[:, b, :], wp[:, blk : blk + P], ident[:]
                        )
                    # PSUM->SBUF copy; alternate engines so consecutive
                    # batches don't serialize
                    dst = wtf[:, bt0 + q0 : bt0 + q0 + qn, :]
                    if (bt0 + q0) // QCT % 2 == 0:
                        nc.scalar.copy(dst, pt[:, :qn, :])
                    else:
                        nc.vector.tensor_copy(dst, pt[:, :qn, :])
            wtf32.append(wtf)

        # ---- stage 3: y.T[rt] = W @ x.T, single fp16 product (f32 PSUM) ----
        for rt in range(RT):
            yp = psum_y.tile([P, N], F32, space="PSUM", tag="yp")
            ct_order = [
                c0 // P + b for c0, width in PIECES for b in range(width // P)
            ]
            for k, ct in enumerate(ct_order):
                nc.tensor.matmul(
                    yp[:],
                    wtf32[rt][:, ct, :],
                    xf[:, ct, :],
                    start=(k == 0),
                    stop=(k == CT - 1),
                )
            ysb = ypool.tile([P, N], F32, tag="ysb")
            nc.scalar.copy(ysb[:], yp[:])
            nc.sync.dma_start(yt_d[rt], ysb[:])

    nc.compile()
    return nc


_PROGRAM = None
_NEFF_CACHE_DIR = os.path.expanduser("~/.cache/bass_neff")


def _install_neff_disk_cache():
    """Cache the walrus NEFF on disk keyed by BIR hash (the walrus compile
    is ~3.5 min; everything else in a fresh process is seconds)."""
    import hashlib

    import concourse.bass2jax as b2j

    if getattr(b2j.compile_bir_kernel, "_disk_cached", False):
        return
    orig = b2j.compile_bir_kernel

    def cached(bir_json, tmpdir, neff_name="file.neff"):
        # the BIR embeds this file's absolute path in DebugInfo; canonicalize
        # it so the cache key is stable across directories
        canon = bir_json.replace(
            os.path.abspath(__file__).encode(), b"@KERNEL@"
        )
        key = hashlib.sha256(canon).hexdigest()[:32]
        path = os.path.join(_NEFF_CACHE_DIR, f"{key}.neff")
        out = os.path.join(tmpdir, neff_name)
        if os.path.exists(path):
            import shutil

            shutil.copy(path, out)
            return out
        neff_file = orig(bir_json, tmpdir, neff_name=neff_name)
        try:
            os.makedirs(_NEFF_CACHE_DIR, exist_ok=True)
            tmp = path + ".tmp"
            import shutil

            shutil.copy(neff_file, tmp)
            os.replace(tmp, path)
        except OSError:
            pass
        return neff_file

    cached._disk_cached = True
    b2j.compile_bir_kernel = cached


def _get_program():
    global _PROGRAM
    if _PROGRAM is None:
        _install_neff_disk_cache()
        _PROGRAM = build_program()
    return _PROGRAM


def make_in_maps(x, data, indices):
    """Host-side layout prep + sharding. No reference arithmetic happens here."""
    x = np.asarray(x, dtype=np.float32)
    data = np.asarray(data, dtype=np.float32)
    indices = np.asarray(indices)

    # x.T tiled [p, ct, n] with c = ct*128 + p, quantized to fp16
    xt = np.ascontiguousarray(
        x.T.reshape(CT, P, N).transpose(1, 0, 2).astype(np.float16)
    )

    ident = np.eye(P, dtype=np.float16)
    vals_all = data.reshape(OUT, J)
    cols_all = indices.reshape(OUT, J).astype(np.float32)

    in_maps = []
    for core in range(NCORES):
        r0 = core * R_PER_CORE
        v = vals_all[r0 : r0 + R_PER_CORE].reshape(RT, P, J).transpose(1, 0, 2)
        c = cols_all[r0 : r0 + R_PER_CORE].reshape(RT, P, J).transpose(1, 0, 2)
        cv = np.ascontiguousarray(np.stack([c, v], axis=1))  # [P, 2, RT, J]
        in_maps.append({"xt": xt, "ident": ident, "cv": cv})
    return in_maps


def kernel(x, data, indices, indptr):
    nc = _get_program()
    in_maps = make_in_maps(x, data, indices)
    res = run_bass_kernel_spmd(nc, in_maps, core_ids=list(range(NCORES)))
    yt = np.concatenate(
        [np.asarray(res.results[c]["yt"]).reshape(R_PER_CORE, N) for c in range(NCORES)],
        axis=0,
    )  # [OUT, N] == y.T
    return np.ascontiguousarray(yt.T.astype(np.float32))


# revision 41
# speedup vs baseline: 6.9123x; 6.9123x over previous
"""CuPyLinear (sparse CSR y = x @ W.T) Trainium2 kernel.

Problem shapes (hardcoded per spec):
  x       [512, 2048] f32
  data    [262144]    f32   (2048 rows x 128 nnz/row, uniform)
  indices [262144]    i32   (sorted per row, duplicates sum)
  indptr  [2049]      i32   (= arange*128, uniform -> unused on device)
  out y   [512, 2048] f32

Sharding: replicate x, shard the 2048 output rows across 8 cores
(256 rows each). Per core:
  1. segmented-scan dedupe of sorted per-row indices (duplicates sum)
  2. densify W rows in fp16 via gpsimd local_scatter (three pieces per
     row tile, each in its own tile for precise dependencies)
  3. transpose W.T with PE identity matmuls (batched through fp16 PSUM,
     PSUM->SBUF copies alternating ACT/DVE)
  4. y.T = W @ x.T as one fp16 matmul set per row tile (f32 PSUM accum)
     End-to-end rel err ~3e-4 (fp16 quantization of W and x).
Host gathers the 8 row-shards of y.T and transposes.
"""

import os
import sys

sys.path.insert(0, "/opt/trn_rl_repo")

from contextlib import ExitStack

import ml_dtypes
import numpy as np

import concourse.bass as bass
import concourse.tile as tile
from concourse import bacc, mybir
from concourse.bass_utils import run_bass_kernel_spmd

P = 128          # partitions
OUT = 2048       # out features (rows of sparse W)
IN = 2048        # in features (cols of sparse W)
N = 512          # tokens
J = 128          # nnz per row (uniform)
NCORES = 8
R_PER_CORE = OUT // NCORES   # 256
RT = R_PER_CORE // P         # 2 row-tiles per core
CT = IN // P                 # 16 contraction tiles
# W is scattered in three pieces per row tile, ordered so the first piece
# has the shortest index-computation path (subtract only) and the last
# piece is small (short critical tail). local_scatter num_elems < 2048.
PIECES = ((1536, 512), (0, 1024), (1024, 512))

BF16 = ml_dtypes.bfloat16
F32 = mybir.dt.float32
BF = mybir.dt.bfloat16
FP16 = mybir.dt.float16
I16 = mybir.dt.int16


def build_program():
    """Build + compile the per-core Bass program (same program on all cores)."""
    nc = bacc.Bacc("TRN2", target_bir_lowering=False, debug=False)

    xt_d = nc.dram_tensor("xt", [P, CT, N], FP16, kind="ExternalInput").ap()
    ident_d = nc.dram_tensor("ident", [P, P], FP16, kind="ExternalInput").ap()
    cv_d = nc.dram_tensor("cv", [P, 2, RT, J], F32, kind="ExternalInput").ap()
    yt_d = nc.dram_tensor("yt", [RT, P, N], F32, kind="ExternalOutput").ap()

    with tile.TileContext(nc) as tc, ExitStack() as ctx:
        const = ctx.enter_context(tc.tile_pool(name="const", bufs=1))
        xpool = ctx.enter_context(tc.tile_pool(name="x", bufs=1))
        work = ctx.enter_context(tc.tile_pool(name="work", bufs=2))
        wpool = ctx.enter_context(tc.tile_pool(name="w", bufs=2))
        psum_t = ctx.enter_context(tc.tile_pool(name="psum_t", bufs=4, space="PSUM"))
        psum_w = ctx.enter_context(tc.tile_pool(name="psum_w", bufs=2, space="PSUM"))
        psum_y = ctx.enter_context(tc.tile_pool(name="psum_y", bufs=2, space="PSUM"))
        ypool = ctx.enter_context(tc.tile_pool(name="y", bufs=2))

        # resident dedupe inputs first so DVE/Pool work starts immediately;
        # the big x tiles stream in behind them.
        cv_sb = xpool.tile([P, 2, RT, J], F32)
        nc.sync.dma_start(cv_sb[:], cv_d[:])
        ident = const.tile([P, P], FP16)
        nc.sync.dma_start(ident[:], ident_d[:])
        xf = xpool.tile([P, CT, N], FP16)
        XCHUNK = CT // 4
        # chunk order matches matmul ct consumption order (piece C first)
        for xc in (12, 0, 4, 8):
            nc.sync.dma_start(
                xf[:, xc : xc + XCHUNK, :], xt_d[:, xc : xc + XCHUNK, :]
            )

        # PE p-state warmup: the tensor engine needs ~3us of continuous
        # work to reach full clock. Chained dummy transposes of the identity
        # keep it busy from when `ident` lands until the first real
        # transposes are ready, so real work runs warm from the start.
        for _ in range(24):
            warm = psum_w.tile([P, P], FP16, space="PSUM", tag="warm")
            nc.tensor.transpose(warm[:], ident[:], ident[:])

        # ---- stage 1: dedupe (segmented scan over sorted cols), one pass
        # per row tile ([128, J] ops; rt0's chain finishes sooner and rt1's
        # overlaps rt0's scatters). Per-rt tiles keep dependencies precise.
        negone = const.tile([P, J], F32)
        nc.vector.memset(negone[:], -1.0)
        s16s = []
        piece_idxs = []
        for rt in range(RT):
            j0 = rt * J
            C = cv_sb[:, 0, rt, :]
            V = cv_sb[:, 1, rt, :]
            # eq[j] = (c[j] == c[j-1]); eq[0] = 0
            eq = work.tile([P, J], F32, tag=f"eq{rt}")
            nc.vector.memset(eq[:, 0:1], 0.0)
            nc.vector.tensor_tensor(
                eq[:, 1:J], C[:, 1:J], C[:, 0 : J - 1], op=mybir.AluOpType.is_equal
            )
            # segmented inclusive sum: s[j] = eq[j]*s[j-1] + v[j]
            s = work.tile([P, J], F32, tag=f"s{rt}")
            nc.vector.tensor_tensor_scan(
                s[:], eq[:], V, 0.0,
                op0=mybir.AluOpType.mult, op1=mybir.AluOpType.add,
            )
            # islast[j] = (c[j] != c[j+1]); islast[J-1] = 1
            islast = work.tile([P, J], mybir.dt.uint8, tag=f"il{rt}")
            nc.vector.memset(islast[:, J - 1 : J], 1)
            nc.vector.tensor_tensor(
                islast[:, 0 : J - 1], C[:, 0 : J - 1], C[:, 1:J],
                op=mybir.AluOpType.not_equal,
            )
            # keep col index only at last-of-run, else -1
            idxk = work.tile([P, J], F32, tag=f"idxk{rt}")
            nc.vector.select(idxk[:], islast[:], C, negone[:])
            # per-piece indices: keep idx-lo when lo <= idx < hi, else
            # negative (ignored by local_scatter)
            pidx = []
            for pi, (lo, width) in enumerate(PIECES):
                hi = lo + width
                if hi < IN:
                    m = work.tile([P, J], mybir.dt.uint8, tag=f"m{pi}_{rt}")
                    nc.vector.tensor_scalar(
                        m[:], idxk[:], float(hi), None, op0=mybir.AluOpType.is_lt
                    )
                    t = work.tile([P, J], F32, tag=f"t{pi}_{rt}")
                    nc.vector.select(t[:], m[:], idxk[:], negone[:])
                else:
                    t = idxk
                ip = work.tile([P, J], I16, tag=f"i{pi}_{rt}")
                if lo > 0:
                    # subtract fused with the int16 cast on the output
                    nc.vector.tensor_scalar_add(ip[:], t[:], -float(lo))
                else:
                    nc.vector.tensor_copy(ip[:], t[:])
                pidx.append(ip)
            piece_idxs.append(pidx)
            # scatter values in fp16 (11-bit mantissa; e2e error ~3e-4)
            s16 = work.tile([P, J], FP16, tag=f"s16{rt}")
            nc.vector.tensor_copy(s16[:], s[:])
            s16s.append(s16)

        # ---- stage 2: densify W rows via local_scatter (r-part, c-free),
        # then PE identity-transposes each piece (which also keeps the PE
        # p-state ramped before the matmuls), 4 [128,128] blocks per fp16
        # PSUM bank, PSUM->SBUF copies alternating ACT/DVE. wtf[pi, po, r]
        # holds W.T row c = po*128 + pi -> the [c-part, ct, r] lhsT layout.
        QCT = 1  # blocks per PSUM copy chunk (smaller -> lower piece latency)
        wtf32 = []
        prev_scatter = None
        from concourse.tile import add_dep_helper
        for rt in range(RT):
            j0 = rt * J
            wtf = wpool.tile([P, CT, P], FP16, tag="wtf")
            for pi, ((c0, width), idx) in enumerate(zip(PIECES, piece_idxs[rt])):
                bt0 = c0 // P
                nblk = width // P
                # each piece scatters into its own tile: precise dependency
                # so this piece's transposes start as soon as IT is done
                wp = wpool.tile([P, width], FP16, tag=f"wp{pi}")
                sc = nc.gpsimd.local_scatter(
                    wp[:],
                    s16s[rt][:],
                    idx[:],
                    channels=P,
                    num_elems=width,
                    num_idxs=J,
                )
                # pin Pool order to emission order (so the small final piece
                # gives a short critical tail)
                if prev_scatter is not None:
                    add_dep_helper(sc.ins, prev_scatter.ins, sync=False)
                prev_scatter = sc
                for q0 in range(0, nblk, QCT):
                    qn = min(QCT, nblk - q0)
                    pt = psum_t.tile([P, QCT, P], FP16, space="PSUM", tag="pt")
                    for b in range(qn):
                        blk = (q0 + b) * P
                        nc.tensor.transpose(
                            pt[:, b, :], wp[:, blk : blk + P], ident[:]
                        )
                    # PSUM->SBUF copy; alternate engines so consecutive
                    # chunks overlap instead of serializing on one engine
                    dst = wtf[:, bt0 + q0 : bt0 + q0 + qn, :]
                    if (bt0 + q0) // QCT % 2 == 0:
                        nc.scalar.copy(dst, pt[:, :qn, :])
                    else:
                        nc.vector.tensor_copy(dst, pt[:, :qn, :])
            wtf32.append(wtf)

        # ---- stage 3: y.T[rt] = W @ x.T, single fp16 product (f32 PSUM) ----
        for rt in range(RT):
            yp = psum_y.tile([P, N], F32, space="PSUM", tag="yp")
            ct_order = [
                c0 // P + b for c0, width in PIECES for b in range(width // P)
            ]
            for k, ct in enumerate(ct_order):
                nc.tensor.matmul(
                    yp[:],
                    wtf32[rt][:, ct, :],
                    xf[:, ct, :],
                    start=(k == 0),
                    stop=(k == CT - 1),
                )
            ysb = ypool.tile([P, N], F32, tag="ysb")
            nc.scalar.copy(ysb[:], yp[:])
            nc.sync.dma_start(yt_d[rt], ysb[:])

    nc.compile()
    return nc


_PROGRAM = None
_NEFF_CACHE_DIR = os.path.expanduser("~/.cache/bass_neff")


def _install_neff_disk_cache():
    """Cache the walrus NEFF on disk keyed by BIR hash (the walrus compile
    is ~3.5 min; everything else in a fresh process is seconds)."""
    import hashlib

    import concourse.bass2jax as b2j

    if getattr(b2j.compile_bir_kernel, "_disk_cached", False):
        return
    orig = b2j.compile_bir_kernel

    def cached(bir_json, tmpdir, neff_name="file.neff"):
        # the BIR embeds this file's absolute path in DebugInfo; canonicalize
        # it so the cache key is stable across directories
        canon = bir_json.replace(
            os.path.abspath(__file__).encode(), b"@KERNEL@"
        )
        key = hashlib.sha256(canon).hexdigest()[:32]
        path = os.path.join(_NEFF_CACHE_DIR, f"{key}.neff")
        out = os.path.join(tmpdir, neff_name)
        if os.path.exists(path):
            import shutil

            shutil.copy(path, out)
            return out
        neff_file = orig(bir_json, tmpdir, neff_name=neff_name)
        try:
            os.makedirs(_NEFF_CACHE_DIR, exist_ok=True)
            tmp = path + ".tmp"
            import shutil

            shutil.copy(neff_file, tmp)
            os.replace(tmp, path)
        except OSError:
            pass
        return neff_file

    cached._disk_cached = True
    b2j.compile_bir_kernel = cached


def _get_program():
    global _PROGRAM
    if _PROGRAM is None:
        _install_neff_disk_cache()
        _PROGRAM = build_program()
    return _PROGRAM


def make_in_maps(x, data, indices):
    """Host-side layout prep + sharding. No reference arithmetic happens here."""
    x = np.asarray(x, dtype=np.float32)
    data = np.asarray(data, dtype=np.float32)
    indices = np.asarray(indices)

    # x.T tiled [p, ct, n] with c = ct*128 + p, quantized to fp16
    xt = np.ascontiguousarray(
        x.T.reshape(CT, P, N).transpose(1, 0, 2).astype(np.float16)
    )

    ident = np.eye(P, dtype=np.float16)
    vals_all = data.reshape(OUT, J)
    cols_all = indices.reshape(OUT, J).astype(np.float32)

    in_maps = []
    for core in range(NCORES):
        r0 = core * R_PER_CORE
        v = vals_all[r0 : r0 + R_PER_CORE].reshape(RT, P, J).transpose(1, 0, 2)
        c = cols_all[r0 : r0 + R_PER_CORE].reshape(RT, P, J).transpose(1, 0, 2)
        cv = np.ascontiguousarray(np.stack([c, v], axis=1))  # [P, 2, RT, J]
        in_maps.append({"xt": xt, "ident": ident, "cv": cv})
    return in_maps


def kernel(x, data, indices, indptr):
    nc = _get_program()
    in_maps = make_in_maps(x, data, indices)
    res = run_bass_kernel_spmd(nc, in_maps, core_ids=list(range(NCORES)))
    yt = np.concatenate(
        [np.asarray(res.results[c]["yt"]).reshape(R_PER_CORE, N) for c in range(NCORES)],
        axis=0,
    )  # [OUT, N] == y.T
    return np.ascontiguousarray(yt.T.astype(np.float32))
